# revision 1
# baseline (speedup 1.0000x reference)
"""Triplane embedding-lookup + MLP kernel for Trainium2 (8 NeuronCores).

Strategy:
  - Host: build a "patch table" PT[3*512*512, 128] where row (pl,y,x) holds the
    4 bilinear-corner pixel vectors [p(y,x), p(y,x+1), p(y+1,x), p(y+1,x+1)]
    (32 channels each). One indirect-DMA descriptor then fetches all data a
    point needs from one plane.
  - Shard the N=1M points across 8 cores (data parallel, planes replicated).
  - Device, per block of 128*K points: compute integer cell ids + bilinear
    weights on DVE/Pool/ACT, one indirect DMA gather (idx [128, 3K]) from PT,
    weighted-sum combine to feats[128, K*32], PE transpose to [32, pts],
    4-layer MLP on PE (bf16), result [1, pts] DMA'd to DRAM.
"""

import sys

sys.path.insert(0, "/opt/trn_rl_repo")

from contextlib import ExitStack

import numpy as np

RES = 512
CELLS = RES * RES
EMB = 32
HID = 128
N = 1_000_000
NCORES = 8

K = 32          # points per partition per block
KT = 992        # points per partition per core (31 blocks of K)
NBLK = KT // K
NP = 128 * KT   # 126976 points per core
BATCH = 4       # k-groups per MLP batch -> 512 points per matmul
NBATCH = K // BATCH

# plane -> (x_coord_index, y_coord_index); x indexes W, y indexes H
PAIRS = ((0, 1), (1, 2), (0, 2))

TABLE_F32 = True   # patch table + combine precision
SEG_GATHER = False  # HW-refuted: no segment<->offset pairing; keep [P,1] gathers
SEGW = 128
LAST_RESULTS = None  # BassKernelResults of the most recent run (for test harness)

_BUILT = {}


def _build_nc(table_dt_name: str, kt: int = KT, do_finalize: bool = True):
    from concourse import bacc, bass, mybir
    import concourse.tile as tile
    from concourse.masks import make_identity

    dt = mybir.dt
    tdt = getattr(dt, table_dt_name)
    f32 = dt.float32
    i32 = dt.int32
    bf16 = dt.bfloat16
    mult = mybir.AluOpType.mult
    add = mybir.AluOpType.add
    AF = mybir.ActivationFunctionType

    nc = bacc.Bacc("TRN2", target_bir_lowering=False)

    ptd = nc.dram_tensor("pt", [3 * CELLS, 128], tdt, kind="ExternalInput")
    crd = nc.dram_tensor("coords", [128 * kt, 3], f32, kind="ExternalInput")
    w0d = nc.dram_tensor("w0t", [EMB, HID], bf16, kind="ExternalInput")
    w1d = nc.dram_tensor("w1t", [HID, HID], bf16, kind="ExternalInput")
    w2d = nc.dram_tensor("w2t", [HID, HID], bf16, kind="ExternalInput")
    w3d = nc.dram_tensor("w3t", [HID, 1], bf16, kind="ExternalInput")
    b0d = nc.dram_tensor("b0c", [HID, 1], f32, kind="ExternalInput")
    b1d = nc.dram_tensor("b1c", [HID, 1], f32, kind="ExternalInput")
    b2d = nc.dram_tensor("b2c", [HID, 1], f32, kind="ExternalInput")
    b3d = nc.dram_tensor("b3c", [1, 1], f32, kind="ExternalInput")
    outd = nc.dram_tensor("out", [kt * 128], f32, kind="ExternalOutput")

    crd3 = crd[:].rearrange("(p kt) c -> p (kt c)", p=128)
    outv = outd[:].unsqueeze(0)

    with tile.TileContext(nc) as tc, ExitStack() as ctx:
        cpool = ctx.enter_context(tc.tile_pool(name="consts", bufs=1))

        def const_tile(shape, dtp, tag):
            return cpool.tile(shape, dtp, tag=tag, name=tag)

        w0s = const_tile([EMB, HID], bf16, "w0s")
        w1s = const_tile([HID, HID], bf16, "w1s")
        w2s = const_tile([HID, HID], bf16, "w2s")
        w3s = const_tile([HID, 1], bf16, "w3s")
        b0s = const_tile([HID, 1], f32, "b0s")
        b1s = const_tile([HID, 1], f32, "b1s")
        b2s = const_tile([HID, 1], f32, "b2s")
        b3s = const_tile([1, 1], f32, "b3s")
        ident = const_tile([128, 128], bf16, "ident")
        for s, d in ((w0s, w0d), (w1s, w1d), (w2s, w2d), (w3s, w3d),
                     (b0s, b0d), (b1s, b1d), (b2s, b2d), (b3s, b3d)):
            nc.sync.dma_start(s[:], d[:])
        make_identity(nc, ident[:])

        work = ctx.enter_context(tc.tile_pool(name="work", bufs=2))
        gpool = ctx.enter_context(tc.tile_pool(name="gather", bufs=2))
        psum = ctx.enter_context(tc.tile_pool(name="psum", bufs=2, space="PSUM"))

        def wt(shape, dtp, tag, bufs=2):
            return work.tile(shape, dtp, tag=tag, name=tag, bufs=bufs)

        for b in range(kt // K):
            c = wt([128, K * 3], f32, "c")
            nc.sync.dma_start(c[:], crd3[:, b * K * 3:(b + 1) * K * 3])

            pix = wt([128, K * 3], f32, "pix")
            nc.scalar.activation(pix[:], c[:], AF.Copy, bias=255.5, scale=255.5)
            # HW f32->i32 cast is rint; cast(pix - 0.5) == floor(pix) for
            # non-integer pix (integer pix may give pix-1 with fr=1.0, which is
            # bilinear-equivalent).
            pixm = wt([128, K * 3], f32, "pixm")
            nc.scalar.activation(pixm[:], c[:], AF.Copy, bias=255.0, scale=255.5)
            ci = wt([128, K * 3], i32, "ci")
            nc.gpsimd.tensor_copy(ci[:], pixm[:])
            cf = wt([128, K * 3], f32, "cf")
            nc.gpsimd.tensor_copy(cf[:], ci[:])
            fr = wt([128, K * 3], f32, "fr")
            nc.vector.tensor_sub(fr[:], pix[:], cf[:])
            omf = wt([128, K * 3], f32, "omf")
            nc.scalar.activation(omf[:], fr[:], AF.Copy, bias=1.0, scale=-1.0)

            ci3 = ci[:].rearrange("p (k c) -> p k c", c=3)
            fr3 = fr[:].rearrange("p (k c) -> p k c", c=3)
            omf3 = omf[:].rearrange("p (k c) -> p k c", c=3)

            idx = wt([128, 3 * K], i32, "idx")
            idx3 = idx[:].rearrange("p (pl k) -> p pl k", pl=3)
            for pl, (xc, yc) in enumerate(PAIRS):
                if pl == 0:
                    nc.vector.scalar_tensor_tensor(
                        out=idx3[:, 0], in0=ci3[:, :, yc], scalar=RES,
                        in1=ci3[:, :, xc], op0=mult, op1=add)
                else:
                    t1 = wt([128, K], i32, "t1", bufs=3)
                    nc.vector.scalar_tensor_tensor(
                        out=t1[:], in0=ci3[:, :, yc], scalar=RES,
                        in1=ci3[:, :, xc], op0=mult, op1=add)
                    nc.vector.tensor_scalar_add(idx3[:, pl], t1[:], pl * CELLS)

            # corner weights: [(1-fy)(1-fx), (1-fy)fx, fy(1-fx), fy fx]
            wts = wt([128, 3 * 4 * K], f32, "wts")
            wts4 = wts[:].rearrange("p (pl c k) -> p pl c k", pl=3, c=4)
            for pl, (xc, yc) in enumerate(PAIRS):
                fx, fy = fr3[:, :, xc], fr3[:, :, yc]
                gx, gy = omf3[:, :, xc], omf3[:, :, yc]
                eng = nc.vector
                eng.tensor_tensor(out=wts4[:, pl, 0], in0=gy, in1=gx, op=mult)
                eng.tensor_tensor(out=wts4[:, pl, 1], in0=gy, in1=fx, op=mult)
                eng.tensor_tensor(out=wts4[:, pl, 2], in0=fy, in1=gx, op=mult)
                eng.tensor_tensor(out=wts4[:, pl, 3], in0=fy, in1=fx, op=mult)

            # HW pairs one offset with each contiguous dest segment. With a
            # strided (non-contiguous) dest view, one instruction carries all
            # 96 offsets; desc-gen is 994ns/instr + 0.34ns/desc, so this cuts
            # GpSimd time ~96x vs per-[P,1] gathers.
            g = gpool.tile([128, 3 * K * SEGW], tdt, tag="g", name="g")
            gseg = g[:].rearrange("p (s f) -> p s f", f=SEGW)
            if SEG_GATHER:
                nc.gpsimd.indirect_dma_start(
                    out=gseg[:, :, 0:128], out_offset=None, in_=ptd[:],
                    in_offset=_ioa()(ap=idx[:], axis=0))
            else:
                for col in range(3 * K):
                    nc.gpsimd.indirect_dma_start(
                        out=gseg[:, col, 0:128], out_offset=None,
                        in_=ptd[:],
                        in_offset=_ioa()(ap=idx[:, col:col + 1], axis=0))
            g5 = g[:].rearrange("p (pl k f) -> p pl k f", pl=3, f=SEGW)

            # all combine on DVE: Pool stays free for SWDGE desc-gen
            acc = wt([128, K * EMB], f32, "accA")
            acc3 = acc[:].rearrange("p (k f) -> p k f", f=EMB)
            terms = [(pl, cc) for pl in range(3) for cc in range(4)]
            feats = wt([128, K * EMB], bf16, "feats")
            for i, (pl, cc) in enumerate(terms):
                w_b = wts4[:, pl, cc].unsqueeze(2).to_broadcast([128, K, EMB])
                gsl = g5[:, pl, :, cc * EMB:(cc + 1) * EMB]
                if i == 0:
                    nc.vector.tensor_tensor(out=acc3, in0=gsl, in1=w_b, op=mult)
                    continue
                prod = wt([128, K * EMB], f32, "prodA", bufs=2)
                nc.vector.tensor_tensor(
                    out=prod[:].rearrange("p (k f) -> p k f", f=EMB),
                    in0=gsl, in1=w_b, op=mult)
                if i == len(terms) - 1:
                    nc.vector.tensor_add(out=feats[:], in0=acc[:], in1=prod[:])
                else:
                    nc.vector.tensor_add(out=acc[:], in0=acc[:], in1=prod[:])

            # MLP
            for j in range(NBATCH):
                k0 = b * K + j * BATCH
                ftp = psum.tile([EMB, BATCH * 128], bf16, tag="ftp", name="ftp",
                                space="PSUM", bufs=2)
                for kk in range(BATCH):
                    nc.tensor.transpose(
                        out=ftp[:, kk * 128:(kk + 1) * 128],
                        in_=feats[:, (j * BATCH + kk) * EMB:(j * BATCH + kk + 1) * EMB],
                        identity=ident[:])
                fts = wt([EMB, BATCH * 128], bf16, "fts")
                nc.scalar.activation(fts[:], ftp[:], AF.Copy)

                mm0 = psum.tile([HID, BATCH * 128], f32, tag="mm", name="mm",
                                space="PSUM", bufs=3)
                nc.tensor.matmul(out=mm0[:], lhsT=w0s[:], rhs=fts[:],
                                 start=True, stop=True)
                h0 = wt([HID, BATCH * 128], bf16, "h0")
                nc.scalar.activation(h0[:], mm0[:], AF.Relu, bias=b0s[:, 0:1])

                mm1 = psum.tile([HID, BATCH * 128], f32, tag="mm", name="mm",
                                space="PSUM", bufs=3)
                nc.tensor.matmul(out=mm1[:], lhsT=w1s[:], rhs=h0[:],
                                 start=True, stop=True)
                h1 = wt([HID, BATCH * 128], bf16, "h1")
                nc.scalar.activation(h1[:], mm1[:], AF.Relu, bias=b1s[:, 0:1])

                mm2 = psum.tile([HID, BATCH * 128], f32, tag="mm", name="mm",
                                space="PSUM", bufs=3)
                nc.tensor.matmul(out=mm2[:], lhsT=w2s[:], rhs=h1[:],
                                 start=True, stop=True)
                h2 = wt([HID, BATCH * 128], bf16, "h2")
                nc.scalar.activation(h2[:], mm2[:], AF.Relu, bias=b2s[:, 0:1])

                mm3 = psum.tile([1, BATCH * 128], f32, tag="mm3", name="mm3",
                                space="PSUM", bufs=2)
                nc.tensor.matmul(out=mm3[:], lhsT=w3s[:], rhs=h2[:],
                                 start=True, stop=True)
                res = wt([1, BATCH * 128], f32, "res")
                nc.scalar.activation(res[:], mm3[:], AF.Identity,
                                     bias=b3s[0:1, 0:1])
                nc.sync.dma_start(outv[:, k0 * 128:(k0 + BATCH) * 128], res[:])

    if do_finalize:
        nc.finalize()
    return nc


def _ioa():
    from concourse import bass
    return bass.IndirectOffsetOnAxis


def _get_nc():
    key = "float32" if TABLE_F32 else "bfloat16"
    if key not in _BUILT:
        _BUILT[key] = _build_nc(key)
    return _BUILT[key]


def _build_patch_table(planes: np.ndarray, np_dt) -> np.ndarray:
    # planes [3, 32, 512, 512] -> PT [3*512*512, 128]
    p = planes.transpose(0, 2, 3, 1)  # [3, H, W, C]
    pt = np.zeros((3, RES, RES, 4, EMB), dtype=np.float32)
    pt[:, :, :, 0] = p
    pt[:, :, :-1, 1] = p[:, :, 1:]
    pt[:, :-1, :, 2] = p[:, 1:]
    pt[:, :-1, :-1, 3] = p[:, 1:, 1:]
    return np.ascontiguousarray(pt.reshape(3 * CELLS, 4 * EMB)).astype(np_dt)


def kernel(**inputs: np.ndarray) -> np.ndarray:
    global LAST_RESULTS
    import ml_dtypes
    from concourse.bass_utils import run_bass_kernel_spmd

    coords = np.asarray(inputs["coordinates"], dtype=np.float32)
    planes = np.asarray(inputs["planes"], dtype=np.float32)
    bf = ml_dtypes.bfloat16
    np_tdt = np.float32 if TABLE_F32 else bf
    pt = _build_patch_table(planes, np_tdt)
    w0t = np.ascontiguousarray(inputs["w0"].T).astype(bf)
    w1t = np.ascontiguousarray(inputs["w1"].T).astype(bf)
    w2t = np.ascontiguousarray(inputs["w2"].T).astype(bf)
    w3t = np.ascontiguousarray(inputs["w3"].T).astype(bf)
    b0 = np.asarray(inputs["b0"], np.float32).reshape(HID, 1)
    b1 = np.asarray(inputs["b1"], np.float32).reshape(HID, 1)
    b2 = np.asarray(inputs["b2"], np.float32).reshape(HID, 1)
    b3 = np.asarray(inputs["b3"], np.float32).reshape(1, 1)

    n = coords.shape[0]
    coords_pad = np.zeros((NCORES * NP, 3), np.float32)
    coords_pad[:n] = coords

    in_maps = []
    for i in range(NCORES):
        in_maps.append({
            "pt": pt,
            "coords": np.ascontiguousarray(coords_pad[i * NP:(i + 1) * NP]),
            "w0t": w0t, "w1t": w1t, "w2t": w2t, "w3t": w3t,
            "b0c": b0, "b1c": b1, "b2c": b2, "b3c": b3,
        })

    nc = _get_nc()
    LAST_RESULTS = run_bass_kernel_spmd(nc, in_maps, list(range(NCORES)))
    pieces = []
    for i in range(NCORES):
        o = np.asarray(LAST_RESULTS.results[i]["out"], np.float32)
        pieces.append(o.reshape(KT, 128).T.ravel())  # -> point order p*KT+k
    full = np.concatenate(pieces)[:n]
    return full.reshape(1, n, 1).astype(np.float32)



# revision 8
# speedup vs baseline: 1.2555x; 1.2555x over previous
"""Triplane embedding-lookup + MLP kernel for Trainium2 (8 NeuronCores).

Strategy (v2, dma_gather):
  - Host: patch table PT[3*512*512, 128] bf16; row (pl,y,x) = 4 bilinear-corner
    pixel vectors (32ch each). Points are bucketed by (y-band, z-band) into
    8x8=64 groups of 2048 slots per core, so every group's gathers hit a
    <=32768-row band slice of PT -- addressable by dma_gather's int16 indices.
  - Device, per group: compute local cell ids (int16) + bilinear weights,
    3 dma_gather calls (2048 rows each, one SWDGE instr apiece -- ~20x less
    Pool time than per-partition indirect DMA), bf16 weighted combine on DVE,
    4-layer MLP on PE (bf16), out [1, 2048] f32 to DRAM.
  - Host: inverse-permute the per-slot outputs back to input point order.
"""

import sys

sys.path.insert(0, "/opt/trn_rl_repo")

from contextlib import ExitStack

import numpy as np

RES = 512
CELLS = RES * RES
EMB = 32
HID = 128
N = 1_000_000
NCORES = 8

BANDS = 8          # bands per axis
BROWS = RES // BANDS   # 64 rows per band
BCELLS = BROWS * RES   # 32768 cells per band slice (int16-addressable)
NGRP = BANDS * BANDS   # 64 groups per core: (y-band, z-band)
S = 2048           # point slots per group
SJ = S // 128      # 16 free-dim cols per partition (weights layout)
NIC = S // 16      # 128 idx cols per partition (idx layout)
NBT = 4            # MLP batches of 512 points per group
NP_CORE = NGRP * S  # 131072 slots per core

LAST_RESULTS = None
_BUILT = {}

# HW f32->i32 cast is rint => floor(pix) == rint(pix - 0.5), bias 255.0.
# CoreSim casts via numpy truncation => floor(pix) == trunc(pix), bias 255.5.
SIM_MODE = False


def _cast_bias() -> float:
    return 255.5 if SIM_MODE else 255.0


def _build_nc(table_dt_name: str = "bfloat16"):
    from concourse import bacc, bass, mybir, library_config
    import concourse.tile as tile
    from concourse.masks import make_identity

    dt = mybir.dt
    tdt = getattr(dt, table_dt_name)
    f32 = dt.float32
    i32 = dt.int32
    i16 = dt.int16
    bf16 = dt.bfloat16
    mult = mybir.AluOpType.mult
    add = mybir.AluOpType.add
    amax = mybir.AluOpType.max
    amin = mybir.AluOpType.min
    AF = mybir.ActivationFunctionType

    nc = bacc.Bacc("TRN2", target_bir_lowering=False)

    ptd = nc.dram_tensor("pt", [3 * CELLS, 128], tdt, kind="ExternalInput")
    crdw = nc.dram_tensor("crdw", [NGRP, 128, SJ * 3], f32, kind="ExternalInput")
    crdi = nc.dram_tensor("crdi", [NGRP, 128, 3 * NIC], f32, kind="ExternalInput")
    w0d = nc.dram_tensor("w0t", [EMB, HID], bf16, kind="ExternalInput")
    w1d = nc.dram_tensor("w1t", [HID, HID], bf16, kind="ExternalInput")
    w2d = nc.dram_tensor("w2t", [HID, HID], bf16, kind="ExternalInput")
    w3d = nc.dram_tensor("w3t", [HID, 1], bf16, kind="ExternalInput")
    b0d = nc.dram_tensor("b0c", [HID, 1], f32, kind="ExternalInput")
    b1d = nc.dram_tensor("b1c", [HID, 1], f32, kind="ExternalInput")
    b2d = nc.dram_tensor("b2c", [HID, 1], f32, kind="ExternalInput")
    b3d = nc.dram_tensor("b3c", [1, 1], f32, kind="ExternalInput")
    outd = nc.dram_tensor("out", [NP_CORE], f32, kind="ExternalOutput")
    outv = outd[:].unsqueeze(0)

    with tile.TileContext(nc) as tc, ExitStack() as ctx:
        nc.gpsimd.load_library(library_config.mlp)

        cpool = ctx.enter_context(tc.tile_pool(name="consts", bufs=1))

        def const_tile(shape, dtp, tag):
            return cpool.tile(shape, dtp, tag=tag, name=tag)

        w0s = const_tile([EMB, HID], bf16, "w0s")
        w1s = const_tile([HID, HID], bf16, "w1s")
        w2s = const_tile([HID, HID], bf16, "w2s")
        w3s = const_tile([HID, 1], bf16, "w3s")
        b0s = const_tile([HID, 1], f32, "b0s")
        b1s = const_tile([HID, 1], f32, "b1s")
        b2s = const_tile([HID, 1], f32, "b2s")
        b3s = const_tile([1, 1], f32, "b3s")
        ident = const_tile([128, 128], bf16, "ident")
        for s_, d_ in ((w0s, w0d), (w1s, w1d), (w2s, w2d), (w3s, w3d),
                       (b0s, b0d), (b1s, b1d), (b2s, b2d), (b3s, b3d)):
            nc.sync.dma_start(s_[:], d_[:])
        make_identity(nc, ident[:])

        work = ctx.enter_context(tc.tile_pool(name="work", bufs=2))
        gpool = ctx.enter_context(tc.tile_pool(name="gather", bufs=2))
        psum = ctx.enter_context(tc.tile_pool(name="psum", bufs=2, space="PSUM"))

        def wt(shape, dtp, tag, bufs=2):
            return work.tile(shape, dtp, tag=tag, name=tag, bufs=bufs)

        for g in range(NGRP):
            ab, bb = g // BANDS, g % BANDS

            # ---- index path (idx-wrapped layout: point i at (i%16, i//16),
            #      replicated x8 down partitions; cols [x NIC | y NIC | z NIC])
            ct = wt([128, 3 * NIC], f32, "ct")
            nc.sync.dma_start(ct[:], crdi[g])
            pm = wt([128, 4 * NIC], f32, "pm")
            cb = _cast_bias()
            nc.scalar.activation(pm[:, 0:2 * NIC], ct[:, 0:2 * NIC], AF.Copy,
                                 bias=cb, scale=255.5)              # px|py
            nc.scalar.activation(pm[:, 2 * NIC:3 * NIC], ct[:, NIC:2 * NIC],
                                 AF.Copy, bias=cb - 64.0 * ab, scale=255.5)
            nc.scalar.activation(pm[:, 3 * NIC:4 * NIC], ct[:, 2 * NIC:3 * NIC],
                                 AF.Copy, bias=cb - 64.0 * bb, scale=255.5)
            ci = wt([128, 4 * NIC], i32, "ci")
            nc.gpsimd.tensor_copy(ci[:], pm[:])
            civ = ci[:].rearrange("p (s t) -> p s t", s=4)
            idx32 = wt([128, 3 * NIC], i32, "idx32")
            iv = idx32[:].rearrange("p (s t) -> p s t", s=3)
            nc.vector.scalar_tensor_tensor(
                out=iv[:, 0], in0=civ[:, 2], scalar=RES, in1=civ[:, 0],
                op0=mult, op1=add)
            nc.vector.scalar_tensor_tensor(
                out=iv[:, 1], in0=civ[:, 3], scalar=RES, in1=civ[:, 1],
                op0=mult, op1=add)
            nc.vector.scalar_tensor_tensor(
                out=iv[:, 2], in0=civ[:, 3], scalar=RES, in1=civ[:, 0],
                op0=mult, op1=add)
            idx16 = wt([128, 3 * NIC], i16, "idx16")
            nc.vector.tensor_scalar(
                out=idx16[:], in0=idx32[:], scalar1=0, scalar2=BCELLS - 1,
                op0=amax, op1=amin)

            # ---- gathers: one dma_gather per plane from its band slice
            # SWDGE ring caps one DMA at 128 descs/engine (NI/16+1 <= 128),
            # so split each plane's 2048-row gather into 2x1024.
            gts = []
            for pl in range(3):
                base = pl * CELLS + (ab if pl == 0 else bb) * BCELLS
                gt = gpool.tile([128, SJ, 128], tdt, tag=f"g{pl}", name=f"g{pl}")
                half = S // 2
                for hh in range(2):
                    nc.gpsimd.dma_gather(
                        gt[:, hh * (SJ // 2):(hh + 1) * (SJ // 2), :],
                        ptd[base:base + BCELLS, :],
                        idx16[:, pl * NIC + hh * (NIC // 2):
                              pl * NIC + (hh + 1) * (NIC // 2)],
                        half, half, 128)
                gts.append(gt)

            # ---- bilinear weights (p-major layout: point i at (i%128, i//128))
            cw = wt([128, SJ * 3], f32, "cw")
            nc.sync.dma_start(cw[:], crdw[g])
            pix = wt([128, SJ * 3], f32, "pix")
            nc.vector.tensor_scalar(out=pix[:], in0=cw[:], scalar1=255.5,
                                    scalar2=255.5, op0=mult, op1=add)
            pixm = wt([128, SJ * 3], f32, "pixm")
            nc.vector.tensor_scalar(out=pixm[:], in0=cw[:], scalar1=255.5,
                                    scalar2=_cast_bias(), op0=mult, op1=add)
            ciw = wt([128, SJ * 3], i32, "ciw")
            nc.gpsimd.tensor_copy(ciw[:], pixm[:])
            cfw = wt([128, SJ * 3], f32, "cfw")
            nc.gpsimd.tensor_copy(cfw[:], ciw[:])
            fr = wt([128, SJ * 3], bf16, "fr")
            nc.vector.tensor_sub(fr[:], pix[:], cfw[:])
            omf = wt([128, SJ * 3], bf16, "omf")
            nc.vector.tensor_scalar(out=omf[:], in0=fr[:], scalar1=-1.0,
                                    scalar2=1.0, op0=mult, op1=add)
            fr3 = fr[:].rearrange("p (j c) -> p j c", c=3)
            omf3 = omf[:].rearrange("p (j c) -> p j c", c=3)

            # corner order matches PT rows: [(y,x),(y,x+1),(y+1,x),(y+1,x+1)]
            wts4 = wt([128, 3 * SJ * 4], bf16, "wts4")
            w4v = wts4[:].rearrange("p (pl j c) -> p pl j c", pl=3, c=4)
            for pl, (xc, yc) in enumerate(((0, 1), (1, 2), (0, 2))):
                fx, fy = fr3[:, :, xc], fr3[:, :, yc]
                gx, gy = omf3[:, :, xc], omf3[:, :, yc]
                nc.vector.tensor_tensor(out=w4v[:, pl, :, 0], in0=gy, in1=gx, op=mult)
                nc.vector.tensor_tensor(out=w4v[:, pl, :, 1], in0=gy, in1=fx, op=mult)
                nc.vector.tensor_tensor(out=w4v[:, pl, :, 2], in0=fy, in1=gx, op=mult)
                nc.vector.tensor_tensor(out=w4v[:, pl, :, 3], in0=fy, in1=fx, op=mult)

            # ---- combine: per plane prod -> corner-pair tree -> plane sum
            s1 = wt([128, 3 * SJ * EMB], bf16, "s1")
            s1v = s1[:].rearrange("p (pl j e) -> p pl j e", pl=3, e=EMB)
            for pl in range(3):
                prod = wt([128, SJ * 4 * EMB], bf16, f"prod{pl}")
                pv = prod[:].rearrange("p (j c e) -> p j c e", c=4, e=EMB)
                g4 = gts[pl][:].rearrange("p j (c e) -> p j c e", c=4)
                wb = w4v[:, pl].unsqueeze(3).to_broadcast([128, SJ, 4, EMB])
                nc.vector.tensor_tensor(out=pv, in0=g4, in1=wb, op=mult)
                p5 = prod[:].rearrange("p (j h l e) -> p j h l e", h=2, l=2, e=EMB)
                s2 = wt([128, SJ * 2 * EMB], bf16, f"s2_{pl}")
                s2v = s2[:].rearrange("p (j h e) -> p j h e", h=2, e=EMB)
                nc.vector.tensor_tensor(out=s2v, in0=p5[:, :, :, 0],
                                        in1=p5[:, :, :, 1], op=add)
                nc.vector.tensor_tensor(out=s1v[:, pl], in0=s2v[:, :, 0],
                                        in1=s2v[:, :, 1], op=add)
            feats = wt([128, SJ * EMB], bf16, "feats")
            fv = feats[:].rearrange("p (j e) -> p j e", e=EMB)
            nc.vector.tensor_tensor(out=fv, in0=s1v[:, 0], in1=s1v[:, 1], op=add)
            nc.vector.tensor_tensor(out=fv, in0=fv, in1=s1v[:, 2], op=add)

            # ---- MLP (batches of 512 points)
            for bt in range(NBT):
                ftp = psum.tile([EMB, 4 * 128], bf16, tag="ftp", name="ftp",
                                space="PSUM", bufs=2)
                for kk in range(4):
                    nc.tensor.transpose(
                        out=ftp[:, kk * 128:(kk + 1) * 128],
                        in_=feats[:, (bt * 4 + kk) * EMB:(bt * 4 + kk + 1) * EMB],
                        identity=ident[:])
                fts = wt([EMB, 4 * 128], bf16, "fts")
                nc.vector.tensor_copy(fts[:], ftp[:])

                mm0 = psum.tile([HID, 4 * 128], f32, tag="mm", name="mm",
                                space="PSUM", bufs=3)
                nc.tensor.matmul(out=mm0[:], lhsT=w0s[:], rhs=fts[:],
                                 start=True, stop=True)
                h0 = wt([HID, 4 * 128], bf16, "h0")
                nc.scalar.activation(h0[:], mm0[:], AF.Relu, bias=b0s[:, 0:1])

                mm1 = psum.tile([HID, 4 * 128], f32, tag="mm", name="mm",
                                space="PSUM", bufs=3)
                nc.tensor.matmul(out=mm1[:], lhsT=w1s[:], rhs=h0[:],
                                 start=True, stop=True)
                h1 = wt([HID, 4 * 128], bf16, "h1")
                nc.vector.tensor_scalar(out=h1[:], in0=mm1[:],
                                        scalar1=b1s[:, 0:1], scalar2=0.0,
                                        op0=add, op1=amax)

                mm2 = psum.tile([HID, 4 * 128], f32, tag="mm", name="mm",
                                space="PSUM", bufs=3)
                nc.tensor.matmul(out=mm2[:], lhsT=w2s[:], rhs=h1[:],
                                 start=True, stop=True)
                h2 = wt([HID, 4 * 128], bf16, "h2")
                nc.scalar.activation(h2[:], mm2[:], AF.Relu, bias=b2s[:, 0:1])

                mm3 = psum.tile([1, 4 * 128], f32, tag="mm3", name="mm3",
                                space="PSUM", bufs=2)
                nc.tensor.matmul(out=mm3[:], lhsT=w3s[:], rhs=h2[:],
                                 start=True, stop=True)
                res = wt([1, 4 * 128], f32, "res")
                nc.scalar.activation(res[:], mm3[:], AF.Identity,
                                     bias=b3s[0:1, 0:1])
                nc.sync.dma_start(
                    outv[:, g * S + bt * 512:g * S + (bt + 1) * 512], res[:])

    nc.finalize()
    return nc


def _get_nc():
    key = ("bfloat16", SIM_MODE)
    if key not in _BUILT:
        _BUILT[key] = _build_nc("bfloat16")
    return _BUILT[key]


def _build_patch_table(planes: np.ndarray, np_dt) -> np.ndarray:
    # planes [3, 32, 512, 512] -> PT [3*512*512, 128]
    p = planes.transpose(0, 2, 3, 1)  # [3, H, W, C]
    pt = np.zeros((3, RES, RES, 4, EMB), dtype=np.float32)
    pt[:, :, :, 0] = p
    pt[:, :, :-1, 1] = p[:, :, 1:]
    pt[:, :-1, :, 2] = p[:, 1:]
    pt[:, :-1, :-1, 3] = p[:, 1:, 1:]
    return np.ascontiguousarray(pt.reshape(3 * CELLS, 4 * EMB)).astype(np_dt)


def _floor_pix(c: np.ndarray) -> np.ndarray:
    # must match device: ACT (c*255.5 + bias) then int32 cast
    pm = c.astype(np.float32) * np.float32(255.5) + np.float32(_cast_bias())
    if SIM_MODE:
        return pm.astype(np.int32)  # numpy trunc, like CoreSim
    return np.rint(pm).astype(np.int32)  # HW rint


def kernel(**inputs: np.ndarray) -> np.ndarray:
    global LAST_RESULTS
    import ml_dtypes
    from concourse.bass_utils import run_bass_kernel_spmd

    coords = np.asarray(inputs["coordinates"], dtype=np.float32)
    planes = np.asarray(inputs["planes"], dtype=np.float32)
    bf = ml_dtypes.bfloat16
    pt = _build_patch_table(planes, bf)
    w0t = np.ascontiguousarray(inputs["w0"].T).astype(bf)
    w1t = np.ascontiguousarray(inputs["w1"].T).astype(bf)
    w2t = np.ascontiguousarray(inputs["w2"].T).astype(bf)
    w3t = np.ascontiguousarray(inputs["w3"].T).astype(bf)
    b0 = np.asarray(inputs["b0"], np.float32).reshape(HID, 1)
    b1 = np.asarray(inputs["b1"], np.float32).reshape(HID, 1)
    b2 = np.asarray(inputs["b2"], np.float32).reshape(1 * HID, 1)
    b3 = np.asarray(inputs["b3"], np.float32).reshape(1, 1)

    n = coords.shape[0]
    y0 = _floor_pix(coords[:, 1])
    z0 = _floor_pix(coords[:, 2])
    ab = np.clip(y0 >> 6, 0, BANDS - 1)
    bb = np.clip(z0 >> 6, 0, BANDS - 1)
    bucket = (ab * BANDS + bb).astype(np.int64)

    order = np.argsort(bucket, kind="stable")
    sorted_ids = order
    counts = np.bincount(bucket, minlength=NGRP)
    starts = np.concatenate(([0], np.cumsum(counts)))

    # slot_map[core, g, i] = original point id (or -1 for pad)
    slot_map = np.full((NCORES, NGRP, S), -1, dtype=np.int64)
    crdw = np.zeros((NCORES, NGRP, 128, SJ * 3), np.float32)
    crdi = np.zeros((NCORES, NGRP, 128, 3 * NIC), np.float32)

    for g in range(NGRP):
        ids = sorted_ids[starts[g]:starts[g + 1]]
        nk = len(ids)
        assert nk <= NCORES * S, f"bucket {g} overflow: {nk}"
        for c in range(NCORES):
            chunk = ids[c::NCORES]
            m = len(chunk)
            assert m <= S, f"core chunk overflow: {m}"
            slot_map[c, g, :m] = chunk
            if m == 0:
                # fabricate an in-band point (weights harmless, output unused)
                aa, zz = g // BANDS, g % BANDS
                fake = np.array([0.0,
                                 (64 * aa + 32) / 255.5 - 1.0,
                                 (64 * zz + 32) / 255.5 - 1.0], np.float32)
                pts = np.tile(fake, (S, 1))
            else:
                pts = coords[chunk]
                if m < S:
                    pts = np.concatenate(
                        [pts, np.tile(pts[0], (S - m, 1))], axis=0)
            crdw[c, g] = pts.reshape(SJ, 128, 3).transpose(1, 0, 2).reshape(
                128, SJ * 3)
            arr = pts.reshape(NIC, 16, 3).transpose(1, 2, 0).reshape(
                16, 3 * NIC)  # [q, (c, t)]
            crdi[c, g] = np.tile(arr, (8, 1))

    in_maps = []
    for c in range(NCORES):
        in_maps.append({
            "pt": pt,
            "crdw": np.ascontiguousarray(crdw[c]),
            "crdi": np.ascontiguousarray(crdi[c]),
            "w0t": w0t, "w1t": w1t, "w2t": w2t, "w3t": w3t,
            "b0c": b0, "b1c": b1, "b2c": b2, "b3c": b3,
        })

    nc = _get_nc()
    LAST_RESULTS = run_bass_kernel_spmd(nc, in_maps, list(range(NCORES)))

    full = np.zeros(n, np.float32)
    for c in range(NCORES):
        o = np.asarray(LAST_RESULTS.results[c]["out"], np.float32).ravel()
        sm = slot_map[c].ravel()
        valid = sm >= 0
        full[sm[valid]] = o[valid]
    return full.reshape(1, n, 1).astype(np.float32)


# revision 10
# speedup vs baseline: 2.3060x; 1.8367x over previous
"""Triplane embedding-lookup + MLP kernel for Trainium2 (8 NeuronCores).

Strategy (v2, dma_gather):
  - Host: patch table PT[3*512*512, 128] bf16; row (pl,y,x) = 4 bilinear-corner
    pixel vectors (32ch each). Points are bucketed by (y-band, z-band) into
    8x8=64 groups of 2048 slots per core, so every group's gathers hit a
    <=32768-row band slice of PT -- addressable by dma_gather's int16 indices.
  - Device, per group: compute local cell ids (int16) + bilinear weights,
    3 dma_gather calls (2048 rows each, one SWDGE instr apiece -- ~20x less
    Pool time than per-partition indirect DMA), bf16 weighted combine on DVE,
    4-layer MLP on PE (bf16), out [1, 2048] f32 to DRAM.
  - Host: inverse-permute the per-slot outputs back to input point order.
"""

import sys

sys.path.insert(0, "/opt/trn_rl_repo")

from contextlib import ExitStack

import numpy as np

RES = 512
CELLS = RES * RES
EMB = 32
HID = 128
N = 1_000_000
NCORES = 8

BANDS = 8          # bands per axis
BROWS = RES // BANDS   # 64 rows per band
BCELLS = BROWS * RES   # 32768 cells per band slice (int16-addressable)
NGRP = BANDS * BANDS   # 64 groups per core: (y-band, z-band)
S = 2048           # point slots per group
SJ = S // 128      # 16 free-dim cols per partition (weights layout)
NIC = S // 16      # 128 idx cols per partition (idx layout)
NBT = 4            # MLP batches of 512 points per group
NP_CORE = NGRP * S  # 131072 slots per core

LAST_RESULTS = None
_BUILT = {}

# HW f32->i32 cast is rint => floor(pix) == rint(pix - 0.5), bias 255.0.
# CoreSim casts via numpy truncation => floor(pix) == trunc(pix), bias 255.5.
SIM_MODE = False


def _cast_bias() -> float:
    return 255.5 if SIM_MODE else 255.0


def _build_nc(table_dt_name: str = "bfloat16"):
    from concourse import bacc, bass, mybir, library_config
    import concourse.tile as tile
    from concourse.masks import make_identity

    dt = mybir.dt
    tdt = getattr(dt, table_dt_name)
    f32 = dt.float32
    i32 = dt.int32
    i16 = dt.int16
    bf16 = dt.bfloat16
    mult = mybir.AluOpType.mult
    add = mybir.AluOpType.add
    amax = mybir.AluOpType.max
    amin = mybir.AluOpType.min
    AF = mybir.ActivationFunctionType

    # 4 SWDGE queues => 4 Q7 core-pairs generate gather descriptors
    # concurrently (measured 3.6x on HW vs 1 queue).
    nc = bacc.Bacc("TRN2", target_bir_lowering=False, num_swdge_queues=4)

    ptd = nc.dram_tensor("pt", [3 * CELLS, 128], tdt, kind="ExternalInput")
    crdw = nc.dram_tensor("crdw", [NGRP, 128, SJ * 3], f32, kind="ExternalInput")
    crdi = nc.dram_tensor("crdi", [NGRP, 128, 3 * NIC], f32, kind="ExternalInput")
    w0d = nc.dram_tensor("w0t", [EMB, HID], bf16, kind="ExternalInput")
    w1d = nc.dram_tensor("w1t", [HID, HID], bf16, kind="ExternalInput")
    w2d = nc.dram_tensor("w2t", [HID, HID], bf16, kind="ExternalInput")
    w3d = nc.dram_tensor("w3t", [HID, 1], bf16, kind="ExternalInput")
    b0d = nc.dram_tensor("b0c", [HID, 1], f32, kind="ExternalInput")
    b1d = nc.dram_tensor("b1c", [HID, 1], f32, kind="ExternalInput")
    b2d = nc.dram_tensor("b2c", [HID, 1], f32, kind="ExternalInput")
    b3d = nc.dram_tensor("b3c", [1, 1], f32, kind="ExternalInput")
    outd = nc.dram_tensor("out", [NP_CORE], f32, kind="ExternalOutput")
    outv = outd[:].unsqueeze(0)

    with tile.TileContext(nc) as tc, ExitStack() as ctx:
        nc.gpsimd.load_library(library_config.mlp)

        cpool = ctx.enter_context(tc.tile_pool(name="consts", bufs=1))

        def const_tile(shape, dtp, tag):
            return cpool.tile(shape, dtp, tag=tag, name=tag)

        w0s = const_tile([EMB, HID], bf16, "w0s")
        w1s = const_tile([HID, HID], bf16, "w1s")
        w2s = const_tile([HID, HID], bf16, "w2s")
        w3s = const_tile([HID, 1], bf16, "w3s")
        b0s = const_tile([HID, 1], f32, "b0s")
        b1s = const_tile([HID, 1], f32, "b1s")
        b2s = const_tile([HID, 1], f32, "b2s")
        b3s = const_tile([1, 1], f32, "b3s")
        ident = const_tile([128, 128], bf16, "ident")
        for s_, d_ in ((w0s, w0d), (w1s, w1d), (w2s, w2d), (w3s, w3d),
                       (b0s, b0d), (b1s, b1d), (b2s, b2d), (b3s, b3d)):
            nc.sync.dma_start(s_[:], d_[:])
        make_identity(nc, ident[:])

        work = ctx.enter_context(tc.tile_pool(name="work", bufs=2))
        gpool = ctx.enter_context(tc.tile_pool(name="gather", bufs=2))
        psum = ctx.enter_context(tc.tile_pool(name="psum", bufs=2, space="PSUM"))

        def wt(shape, dtp, tag, bufs=2):
            return work.tile(shape, dtp, tag=tag, name=tag, bufs=bufs)

        for g in range(NGRP):
            ab, bb = g // BANDS, g % BANDS

            # ---- index path (idx-wrapped layout: point i at (i%16, i//16),
            #      replicated x8 down partitions; cols [x NIC | y NIC | z NIC])
            ct = wt([128, 3 * NIC], f32, "ct")
            nc.sync.dma_start(ct[:], crdi[g])
            pm = wt([128, 4 * NIC], f32, "pm")
            cb = _cast_bias()
            nc.scalar.activation(pm[:, 0:2 * NIC], ct[:, 0:2 * NIC], AF.Copy,
                                 bias=cb, scale=255.5)              # px|py
            nc.scalar.activation(pm[:, 2 * NIC:3 * NIC], ct[:, NIC:2 * NIC],
                                 AF.Copy, bias=cb - 64.0 * ab, scale=255.5)
            nc.scalar.activation(pm[:, 3 * NIC:4 * NIC], ct[:, 2 * NIC:3 * NIC],
                                 AF.Copy, bias=cb - 64.0 * bb, scale=255.5)
            ci = wt([128, 4 * NIC], i32, "ci")
            nc.gpsimd.tensor_copy(ci[:], pm[:])
            civ = ci[:].rearrange("p (s t) -> p s t", s=4)
            idx32 = wt([128, 3 * NIC], i32, "idx32")
            iv = idx32[:].rearrange("p (s t) -> p s t", s=3)
            nc.vector.scalar_tensor_tensor(
                out=iv[:, 0], in0=civ[:, 2], scalar=RES, in1=civ[:, 0],
                op0=mult, op1=add)
            nc.vector.scalar_tensor_tensor(
                out=iv[:, 1], in0=civ[:, 3], scalar=RES, in1=civ[:, 1],
                op0=mult, op1=add)
            nc.vector.scalar_tensor_tensor(
                out=iv[:, 2], in0=civ[:, 3], scalar=RES, in1=civ[:, 0],
                op0=mult, op1=add)
            idx16 = wt([128, 3 * NIC], i16, "idx16")
            nc.vector.tensor_scalar(
                out=idx16[:], in0=idx32[:], scalar1=0, scalar2=BCELLS - 1,
                op0=amax, op1=amin)

            # ---- gathers: one dma_gather per plane from its band slice
            # SWDGE ring caps one DMA at <128 descs/engine (NI/16+1), so
            # split each plane's 2048-row gather into 2x1024, spread round-
            # robin over the 4 SWDGE queues (parallel Q7 pairs).
            gts = []
            for pl in range(3):
                base = pl * CELLS + (ab if pl == 0 else bb) * BCELLS
                gt = gpool.tile([128, SJ, 128], tdt, tag=f"g{pl}", name=f"g{pl}")
                half = S // 2
                for hh in range(2):
                    nc.gpsimd.dma_gather(
                        gt[:, hh * (SJ // 2):(hh + 1) * (SJ // 2), :],
                        ptd[base:base + BCELLS, :],
                        idx16[:, pl * NIC + hh * (NIC // 2):
                              pl * NIC + (hh + 1) * (NIC // 2)],
                        half, half, 128,
                        queue_num=(g * 6 + pl * 2 + hh) % 4)
                gts.append(gt)

            # ---- bilinear weights (p-major layout: point i at (i%128, i//128))
            cw = wt([128, SJ * 3], f32, "cw")
            nc.sync.dma_start(cw[:], crdw[g])
            pix = wt([128, SJ * 3], f32, "pix")
            nc.vector.tensor_scalar(out=pix[:], in0=cw[:], scalar1=255.5,
                                    scalar2=255.5, op0=mult, op1=add)
            pixm = wt([128, SJ * 3], f32, "pixm")
            nc.vector.tensor_scalar(out=pixm[:], in0=cw[:], scalar1=255.5,
                                    scalar2=_cast_bias(), op0=mult, op1=add)
            ciw = wt([128, SJ * 3], i32, "ciw")
            nc.gpsimd.tensor_copy(ciw[:], pixm[:])
            cfw = wt([128, SJ * 3], f32, "cfw")
            nc.gpsimd.tensor_copy(cfw[:], ciw[:])
            fr = wt([128, SJ * 3], bf16, "fr")
            nc.vector.tensor_sub(fr[:], pix[:], cfw[:])
            omf = wt([128, SJ * 3], bf16, "omf")
            nc.vector.tensor_scalar(out=omf[:], in0=fr[:], scalar1=-1.0,
                                    scalar2=1.0, op0=mult, op1=add)
            fr3 = fr[:].rearrange("p (j c) -> p j c", c=3)
            omf3 = omf[:].rearrange("p (j c) -> p j c", c=3)

            # corner order matches PT rows: [(y,x),(y,x+1),(y+1,x),(y+1,x+1)]
            wts4 = wt([128, 3 * SJ * 4], bf16, "wts4")
            w4v = wts4[:].rearrange("p (pl j c) -> p pl j c", pl=3, c=4)
            for pl, (xc, yc) in enumerate(((0, 1), (1, 2), (0, 2))):
                fx, fy = fr3[:, :, xc], fr3[:, :, yc]
                gx, gy = omf3[:, :, xc], omf3[:, :, yc]
                nc.vector.tensor_tensor(out=w4v[:, pl, :, 0], in0=gy, in1=gx, op=mult)
                nc.vector.tensor_tensor(out=w4v[:, pl, :, 1], in0=gy, in1=fx, op=mult)
                nc.vector.tensor_tensor(out=w4v[:, pl, :, 2], in0=fy, in1=gx, op=mult)
                nc.vector.tensor_tensor(out=w4v[:, pl, :, 3], in0=fy, in1=fx, op=mult)

            # ---- combine: per plane prod -> corner-pair tree -> plane sum
            s1 = wt([128, 3 * SJ * EMB], bf16, "s1")
            s1v = s1[:].rearrange("p (pl j e) -> p pl j e", pl=3, e=EMB)
            for pl in range(3):
                prod = wt([128, SJ * 4 * EMB], bf16, f"prod{pl}")
                pv = prod[:].rearrange("p (j c e) -> p j c e", c=4, e=EMB)
                g4 = gts[pl][:].rearrange("p j (c e) -> p j c e", c=4)
                wb = w4v[:, pl].unsqueeze(3).to_broadcast([128, SJ, 4, EMB])
                nc.vector.tensor_tensor(out=pv, in0=g4, in1=wb, op=mult)
                p5 = prod[:].rearrange("p (j h l e) -> p j h l e", h=2, l=2, e=EMB)
                s2 = wt([128, SJ * 2 * EMB], bf16, f"s2_{pl}")
                s2v = s2[:].rearrange("p (j h e) -> p j h e", h=2, e=EMB)
                nc.vector.tensor_tensor(out=s2v, in0=p5[:, :, :, 0],
                                        in1=p5[:, :, :, 1], op=add)
                nc.vector.tensor_tensor(out=s1v[:, pl], in0=s2v[:, :, 0],
                                        in1=s2v[:, :, 1], op=add)
            feats = wt([128, SJ * EMB], bf16, "feats")
            fv = feats[:].rearrange("p (j e) -> p j e", e=EMB)
            nc.vector.tensor_tensor(out=fv, in0=s1v[:, 0], in1=s1v[:, 1], op=add)
            nc.vector.tensor_tensor(out=fv, in0=fv, in1=s1v[:, 2], op=add)

            # ---- MLP (batches of 512 points)
            for bt in range(NBT):
                ftp = psum.tile([EMB, 4 * 128], bf16, tag="ftp", name="ftp",
                                space="PSUM", bufs=2)
                for kk in range(4):
                    nc.tensor.transpose(
                        out=ftp[:, kk * 128:(kk + 1) * 128],
                        in_=feats[:, (bt * 4 + kk) * EMB:(bt * 4 + kk + 1) * EMB],
                        identity=ident[:])
                fts = wt([EMB, 4 * 128], bf16, "fts")
                nc.vector.tensor_copy(fts[:], ftp[:])

                mm0 = psum.tile([HID, 4 * 128], f32, tag="mm", name="mm",
                                space="PSUM", bufs=3)
                nc.tensor.matmul(out=mm0[:], lhsT=w0s[:], rhs=fts[:],
                                 start=True, stop=True)
                h0 = wt([HID, 4 * 128], bf16, "h0")
                nc.scalar.activation(h0[:], mm0[:], AF.Relu, bias=b0s[:, 0:1])

                mm1 = psum.tile([HID, 4 * 128], f32, tag="mm", name="mm",
                                space="PSUM", bufs=3)
                nc.tensor.matmul(out=mm1[:], lhsT=w1s[:], rhs=h0[:],
                                 start=True, stop=True)
                h1 = wt([HID, 4 * 128], bf16, "h1")
                nc.vector.tensor_scalar(out=h1[:], in0=mm1[:],
                                        scalar1=b1s[:, 0:1], scalar2=0.0,
                                        op0=add, op1=amax)

                mm2 = psum.tile([HID, 4 * 128], f32, tag="mm", name="mm",
                                space="PSUM", bufs=3)
                nc.tensor.matmul(out=mm2[:], lhsT=w2s[:], rhs=h1[:],
                                 start=True, stop=True)
                h2 = wt([HID, 4 * 128], bf16, "h2")
                nc.scalar.activation(h2[:], mm2[:], AF.Relu, bias=b2s[:, 0:1])

                mm3 = psum.tile([1, 4 * 128], f32, tag="mm3", name="mm3",
                                space="PSUM", bufs=2)
                nc.tensor.matmul(out=mm3[:], lhsT=w3s[:], rhs=h2[:],
                                 start=True, stop=True)
                res = wt([1, 4 * 128], f32, "res")
                nc.scalar.activation(res[:], mm3[:], AF.Identity,
                                     bias=b3s[0:1, 0:1])
                nc.sync.dma_start(
                    outv[:, g * S + bt * 512:g * S + (bt + 1) * 512], res[:])

    nc.finalize()
    return nc


def _get_nc():
    key = ("bfloat16", SIM_MODE)
    if key not in _BUILT:
        _BUILT[key] = _build_nc("bfloat16")
    return _BUILT[key]


def _build_patch_table(planes: np.ndarray, np_dt) -> np.ndarray:
    # planes [3, 32, 512, 512] -> PT [3*512*512, 128]
    p = planes.transpose(0, 2, 3, 1)  # [3, H, W, C]
    pt = np.zeros((3, RES, RES, 4, EMB), dtype=np.float32)
    pt[:, :, :, 0] = p
    pt[:, :, :-1, 1] = p[:, :, 1:]
    pt[:, :-1, :, 2] = p[:, 1:]
    pt[:, :-1, :-1, 3] = p[:, 1:, 1:]
    return np.ascontiguousarray(pt.reshape(3 * CELLS, 4 * EMB)).astype(np_dt)


def _floor_pix(c: np.ndarray) -> np.ndarray:
    # must match device: ACT (c*255.5 + bias) then int32 cast
    pm = c.astype(np.float32) * np.float32(255.5) + np.float32(_cast_bias())
    if SIM_MODE:
        return pm.astype(np.int32)  # numpy trunc, like CoreSim
    return np.rint(pm).astype(np.int32)  # HW rint


def kernel(**inputs: np.ndarray) -> np.ndarray:
    global LAST_RESULTS
    import ml_dtypes
    from concourse.bass_utils import run_bass_kernel_spmd

    coords = np.asarray(inputs["coordinates"], dtype=np.float32)
    planes = np.asarray(inputs["planes"], dtype=np.float32)
    bf = ml_dtypes.bfloat16
    pt = _build_patch_table(planes, bf)
    w0t = np.ascontiguousarray(inputs["w0"].T).astype(bf)
    w1t = np.ascontiguousarray(inputs["w1"].T).astype(bf)
    w2t = np.ascontiguousarray(inputs["w2"].T).astype(bf)
    w3t = np.ascontiguousarray(inputs["w3"].T).astype(bf)
    b0 = np.asarray(inputs["b0"], np.float32).reshape(HID, 1)
    b1 = np.asarray(inputs["b1"], np.float32).reshape(HID, 1)
    b2 = np.asarray(inputs["b2"], np.float32).reshape(1 * HID, 1)
    b3 = np.asarray(inputs["b3"], np.float32).reshape(1, 1)

    n = coords.shape[0]
    y0 = _floor_pix(coords[:, 1])
    z0 = _floor_pix(coords[:, 2])
    ab = np.clip(y0 >> 6, 0, BANDS - 1)
    bb = np.clip(z0 >> 6, 0, BANDS - 1)
    bucket = (ab * BANDS + bb).astype(np.int64)

    order = np.argsort(bucket, kind="stable")
    sorted_ids = order
    counts = np.bincount(bucket, minlength=NGRP)
    starts = np.concatenate(([0], np.cumsum(counts)))

    # slot_map[core, g, i] = original point id (or -1 for pad)
    slot_map = np.full((NCORES, NGRP, S), -1, dtype=np.int64)
    crdw = np.zeros((NCORES, NGRP, 128, SJ * 3), np.float32)
    crdi = np.zeros((NCORES, NGRP, 128, 3 * NIC), np.float32)

    for g in range(NGRP):
        ids = sorted_ids[starts[g]:starts[g + 1]]
        nk = len(ids)
        assert nk <= NCORES * S, f"bucket {g} overflow: {nk}"
        for c in range(NCORES):
            chunk = ids[c::NCORES]
            m = len(chunk)
            assert m <= S, f"core chunk overflow: {m}"
            slot_map[c, g, :m] = chunk
            if m == 0:
                # fabricate an in-band point (weights harmless, output unused)
                aa, zz = g // BANDS, g % BANDS
                fake = np.array([0.0,
                                 (64 * aa + 32) / 255.5 - 1.0,
                                 (64 * zz + 32) / 255.5 - 1.0], np.float32)
                pts = np.tile(fake, (S, 1))
            else:
                pts = coords[chunk]
                if m < S:
                    pts = np.concatenate(
                        [pts, np.tile(pts[0], (S - m, 1))], axis=0)
            crdw[c, g] = pts.reshape(SJ, 128, 3).transpose(1, 0, 2).reshape(
                128, SJ * 3)
            arr = pts.reshape(NIC, 16, 3).transpose(1, 2, 0).reshape(
                16, 3 * NIC)  # [q, (c, t)]
            crdi[c, g] = np.tile(arr, (8, 1))

    in_maps = []
    for c in range(NCORES):
        in_maps.append({
            "pt": pt,
            "crdw": np.ascontiguousarray(crdw[c]),
            "crdi": np.ascontiguousarray(crdi[c]),
            "w0t": w0t, "w1t": w1t, "w2t": w2t, "w3t": w3t,
            "b0c": b0, "b1c": b1, "b2c": b2, "b3c": b3,
        })

    nc = _get_nc()
    LAST_RESULTS = run_bass_kernel_spmd(nc, in_maps, list(range(NCORES)))

    full = np.zeros(n, np.float32)
    for c in range(NCORES):
        o = np.asarray(LAST_RESULTS.results[c]["out"], np.float32).ravel()
        sm = slot_map[c].ravel()
        valid = sm >= 0
        full[sm[valid]] = o[valid]
    return full.reshape(1, n, 1).astype(np.float32)


# revision 16
# speedup vs baseline: 2.9858x; 1.2948x over previous
"""Triplane embedding-lookup + MLP kernel for Trainium2 (8 NeuronCores).

Strategy (v2, dma_gather):
  - Host: patch table PT[3*512*512, 128] bf16; row (pl,y,x) = 4 bilinear-corner
    pixel vectors (32ch each). Points are bucketed by (y-band, z-band) into
    8x8=64 groups of 2048 slots per core, so every group's gathers hit a
    <=32768-row band slice of PT -- addressable by dma_gather's int16 indices.
  - Device, per group: compute local cell ids (int16) + bilinear weights,
    3 dma_gather calls (2048 rows each, one SWDGE instr apiece -- ~20x less
    Pool time than per-partition indirect DMA), bf16 weighted combine on DVE,
    4-layer MLP on PE (bf16), out [1, 2048] f32 to DRAM.
  - Host: inverse-permute the per-slot outputs back to input point order.
"""

import sys

sys.path.insert(0, "/opt/trn_rl_repo")

from contextlib import ExitStack

import numpy as np

RES = 512
CELLS = RES * RES
EMB = 32
HID = 128
N = 1_000_000
NCORES = 8

BANDS = 8          # bands per axis
BROWS = RES // BANDS   # 64 rows per band
BCELLS = BROWS * RES   # 32768 cells per band slice (int16-addressable)
NGRP = BANDS * BANDS   # 64 groups per core: (y-band, z-band)
S = 2048           # point slots per group
SJ = S // 128      # 16 free-dim cols per partition (weights layout)
NIC = S // 16      # 128 idx cols per partition (idx layout)
NBT = 4            # MLP batches of 512 points per group
NP_CORE = NGRP * S  # 131072 slots per core

LAST_RESULTS = None
_BUILT = {}

# HW f32->i32 cast is rint => floor(pix) == rint(pix - 0.5), bias 255.0.
# CoreSim casts via numpy truncation => floor(pix) == trunc(pix), bias 255.5.
SIM_MODE = False


def _cast_bias() -> float:
    return 255.5 if SIM_MODE else 255.0


def _build_nc(table_dt_name: str = "bfloat16"):
    from concourse import bacc, bass, mybir, library_config
    import concourse.tile as tile
    from concourse.masks import make_identity

    dt = mybir.dt
    tdt = getattr(dt, table_dt_name)
    f32 = dt.float32
    i32 = dt.int32
    i16 = dt.int16
    bf16 = dt.bfloat16
    mult = mybir.AluOpType.mult
    add = mybir.AluOpType.add
    amax = mybir.AluOpType.max
    amin = mybir.AluOpType.min
    AF = mybir.ActivationFunctionType

    # 4 SWDGE queues => 4 Q7 core-pairs generate gather descriptors
    # concurrently (measured 3.6x on HW vs 1 queue).
    nc = bacc.Bacc("TRN2", target_bir_lowering=False, num_swdge_queues=4)

    ptd = nc.dram_tensor("pt", [3 * CELLS, 128], tdt, kind="ExternalInput")
    crdw = nc.dram_tensor("crdw", [NGRP, 128, 6 * SJ], f32, kind="ExternalInput")
    crdi = nc.dram_tensor("crdi", [NGRP, 128, 3 * NIC], f32, kind="ExternalInput")
    w0d = nc.dram_tensor("w0t", [EMB, HID], bf16, kind="ExternalInput")
    w1d = nc.dram_tensor("w1t", [HID, HID], bf16, kind="ExternalInput")
    w2d = nc.dram_tensor("w2t", [HID, HID], bf16, kind="ExternalInput")
    w3d = nc.dram_tensor("w3t", [HID, 1], bf16, kind="ExternalInput")
    b0d = nc.dram_tensor("b0c", [HID, 1], f32, kind="ExternalInput")
    b1d = nc.dram_tensor("b1c", [HID, 1], f32, kind="ExternalInput")
    b2d = nc.dram_tensor("b2c", [HID, 1], f32, kind="ExternalInput")
    b3d = nc.dram_tensor("b3c", [1, 1], f32, kind="ExternalInput")
    outd = nc.dram_tensor("out", [NP_CORE], f32, kind="ExternalOutput")
    outv = outd[:].unsqueeze(0)

    with tile.TileContext(nc) as tc, ExitStack() as ctx:
        nc.gpsimd.load_library(library_config.mlp)

        cpool = ctx.enter_context(tc.tile_pool(name="consts", bufs=1))

        def const_tile(shape, dtp, tag):
            return cpool.tile(shape, dtp, tag=tag, name=tag)

        w0s = const_tile([EMB, HID], bf16, "w0s")
        w1s = const_tile([HID, HID], bf16, "w1s")
        w2s = const_tile([HID, HID], bf16, "w2s")
        w3s = const_tile([HID, 1], bf16, "w3s")
        b0s = const_tile([HID, 1], f32, "b0s")
        b1s = const_tile([HID, 1], f32, "b1s")
        b2s = const_tile([HID, 1], f32, "b2s")
        b3s = const_tile([1, 1], f32, "b3s")
        ident = const_tile([128, 128], bf16, "ident")
        for s_, d_ in ((w0s, w0d), (w1s, w1d), (w2s, w2d), (w3s, w3d),
                       (b0s, b0d), (b1s, b1d), (b2s, b2d), (b3s, b3d)):
            nc.sync.dma_start(s_[:], d_[:])
        make_identity(nc, ident[:])

        work = ctx.enter_context(tc.tile_pool(name="work", bufs=2))
        gpool = ctx.enter_context(tc.tile_pool(name="gather", bufs=2))
        psum = ctx.enter_context(tc.tile_pool(name="psum", bufs=2, space="PSUM"))

        def wt(shape, dtp, tag, bufs=2):
            return work.tile(shape, dtp, tag=tag, name=tag, bufs=bufs)

        for g in range(NGRP):
            ab, bb = g // BANDS, g % BANDS

            # ---- index path (idx-wrapped layout: point i at (i%16, i//16),
            #      replicated x8 down partitions; cols [x NIC | y NIC | z NIC])
            ct = wt([128, 3 * NIC], f32, "ct")
            nc.sync.dma_start(ct[:], crdi[g])
            pm = wt([128, 4 * NIC], f32, "pm")
            cb = _cast_bias()
            nc.scalar.activation(pm[:, 0:2 * NIC], ct[:, 0:2 * NIC], AF.Copy,
                                 bias=cb, scale=255.5)              # px|py
            nc.scalar.activation(pm[:, 2 * NIC:3 * NIC], ct[:, NIC:2 * NIC],
                                 AF.Copy, bias=cb - 64.0 * ab, scale=255.5)
            nc.scalar.activation(pm[:, 3 * NIC:4 * NIC], ct[:, 2 * NIC:3 * NIC],
                                 AF.Copy, bias=cb - 64.0 * bb, scale=255.5)
            ci = wt([128, 4 * NIC], i32, "ci")
            nc.scalar.activation(ci[:], pm[:], AF.Copy)  # f32->i32 rint on HW
            civ = ci[:].rearrange("p (s t) -> p s t", s=4)
            idx32 = wt([128, 3 * NIC], i32, "idx32")
            iv = idx32[:].rearrange("p (s t) -> p s t", s=3)
            # planes 0,1 fused: (yA,zB)*512 + (px,py)
            nc.vector.scalar_tensor_tensor(
                out=idx32[:, 0:2 * NIC], in0=ci[:, 2 * NIC:4 * NIC], scalar=RES,
                in1=ci[:, 0:2 * NIC], op0=mult, op1=add)
            nc.vector.scalar_tensor_tensor(
                out=iv[:, 2], in0=civ[:, 3], scalar=RES, in1=civ[:, 0],
                op0=mult, op1=add)
            idx16 = wt([128, 3 * NIC], i16, "idx16")
            nc.vector.tensor_scalar(
                out=idx16[:], in0=idx32[:], scalar1=0, scalar2=BCELLS - 1,
                op0=amax, op1=amin)

            # ---- gathers: one dma_gather per plane from its band slice
            # SWDGE ring caps one DMA at <128 descs/engine (NI/16+1), so
            # split each plane's 2048-row gather into 2x1024, spread round-
            # robin over the 4 SWDGE queues (parallel Q7 pairs).
            gts = []
            for pl in range(3):
                base = pl * CELLS + (ab if pl == 0 else bb) * BCELLS
                gt = gpool.tile([128, SJ, 128], tdt, tag=f"g{pl}", name=f"g{pl}")
                half = S // 2
                for hh in range(2):
                    nc.gpsimd.dma_gather(
                        gt[:, hh * (SJ // 2):(hh + 1) * (SJ // 2), :],
                        ptd[base:base + BCELLS, :],
                        idx16[:, pl * NIC + hh * (NIC // 2):
                              pl * NIC + (hh + 1) * (NIC // 2)],
                        half, half, 128,
                        queue_num=(g * 6 + pl * 2 + hh) % 4)
                gts.append(gt)

            # ---- bilinear weights (p-major layout: point i at (i%128, i//128))
            # cw streams (pl-major): [y(pl0) y(pl1) y(pl2) x(pl0) x(pl1) x(pl2)]
            cw = wt([128, 6 * SJ], f32, "cw")
            nc.sync.dma_start(cw[:], crdw[g])
            pix = wt([128, 6 * SJ], f32, "pix")
            nc.scalar.activation(pix[:], cw[:], AF.Copy, bias=255.5, scale=255.5)
            pixm = wt([128, 6 * SJ], f32, "pixm")
            nc.scalar.activation(pixm[:], cw[:], AF.Copy, bias=_cast_bias(),
                                 scale=255.5)
            ciw = wt([128, 6 * SJ], i32, "ciw")
            nc.scalar.activation(ciw[:], pixm[:], AF.Copy)
            cfw = wt([128, 6 * SJ], f32, "cfw")
            nc.scalar.activation(cfw[:], ciw[:], AF.Copy)
            # frp[p, s, j, k]: k=0 -> 1-frac, k=1 -> frac  (s = 6 streams)
            frp = wt([128, 6 * SJ * 2], bf16, "frp")
            frv = frp[:].rearrange("p (s j k) -> p s j k", s=6, k=2)
            nc.vector.tensor_sub(
                frv[:, :, :, 1],
                pix[:].rearrange("p (s j) -> p s j", s=6),
                cfw[:].rearrange("p (s j) -> p s j", s=6))
            nc.vector.tensor_scalar(out=frv[:, :, :, 0], in0=frv[:, :, :, 1],
                                    scalar1=-1.0, scalar2=1.0,
                                    op0=mult, op1=add)
            # corner weights: W[p, pl, j, ky, kx] = wy[ky] * wx[kx]; corner
            # order matches PT rows [(y,x),(y,x+1),(y+1,x),(y+1,x+1)]
            wts4 = wt([128, 3 * SJ * 4], bf16, "wts4")
            wv5 = wts4[:].rearrange("p (pl j a b) -> p pl j a b", pl=3, a=2, b=2)
            nc.vector.tensor_tensor(
                out=wv5,
                in0=frv[:, 0:3].unsqueeze(4).to_broadcast([128, 3, SJ, 2, 2]),
                in1=frv[:, 3:6].unsqueeze(3).to_broadcast([128, 3, SJ, 2, 2]),
                op=mult)
            w4v = wts4[:].rearrange("p (pl j c) -> p pl j c", pl=3, c=4)

            # ---- combine: per plane prod -> corner-pair tree -> plane sum
            s1 = wt([128, 3 * SJ * EMB], bf16, "s1")
            s1v = s1[:].rearrange("p (pl j e) -> p pl j e", pl=3, e=EMB)
            for pl in range(3):
                prod = wt([128, SJ * 4 * EMB], bf16, f"prod{pl}")
                pv = prod[:].rearrange("p (j c e) -> p j c e", c=4, e=EMB)
                g4 = gts[pl][:].rearrange("p j (c e) -> p j c e", c=4)
                wb = w4v[:, pl].unsqueeze(3).to_broadcast([128, SJ, 4, EMB])
                nc.vector.tensor_tensor(out=pv, in0=g4, in1=wb, op=mult)
                p5 = prod[:].rearrange("p (j h l e) -> p j h l e", h=2, l=2, e=EMB)
                s2 = wt([128, SJ * 2 * EMB], bf16, f"s2_{pl}")
                s2v = s2[:].rearrange("p (j h e) -> p j h e", h=2, e=EMB)
                nc.vector.tensor_tensor(out=s2v, in0=p5[:, :, :, 0],
                                        in1=p5[:, :, :, 1], op=add)
                nc.vector.tensor_tensor(out=s1v[:, pl], in0=s2v[:, :, 0],
                                        in1=s2v[:, :, 1], op=add)
            feats = wt([128, SJ * EMB], bf16, "feats")
            fv = feats[:].rearrange("p (j e) -> p j e", e=EMB)
            nc.vector.tensor_tensor(out=fv, in0=s1v[:, 0], in1=s1v[:, 1], op=add)
            nc.vector.tensor_tensor(out=fv, in0=fv, in1=s1v[:, 2], op=add)

            # ---- MLP (batches of 512 points)
            for bt in range(NBT):
                ftp = psum.tile([EMB, 4 * 128], bf16, tag="ftp", name="ftp",
                                space="PSUM", bufs=2)
                for kk in range(4):
                    nc.tensor.transpose(
                        out=ftp[:, kk * 128:(kk + 1) * 128],
                        in_=feats[:, (bt * 4 + kk) * EMB:(bt * 4 + kk + 1) * EMB],
                        identity=ident[:])
                fts = wt([EMB, 4 * 128], bf16, "fts")
                nc.scalar.activation(fts[:], ftp[:], AF.Copy)

                mm0 = psum.tile([HID, 4 * 128], f32, tag="mm", name="mm",
                                space="PSUM", bufs=3)
                nc.tensor.matmul(out=mm0[:], lhsT=w0s[:], rhs=fts[:],
                                 start=True, stop=True)
                h0 = wt([HID, 4 * 128], bf16, "h0")
                nc.scalar.activation(h0[:], mm0[:], AF.Relu, bias=b0s[:, 0:1])

                mm1 = psum.tile([HID, 4 * 128], f32, tag="mm", name="mm",
                                space="PSUM", bufs=3)
                nc.tensor.matmul(out=mm1[:], lhsT=w1s[:], rhs=h0[:],
                                 start=True, stop=True)
                h1 = wt([HID, 4 * 128], bf16, "h1")
                nc.vector.tensor_scalar(out=h1[:], in0=mm1[:],
                                        scalar1=b1s[:, 0:1], scalar2=0.0,
                                        op0=add, op1=amax)

                mm2 = psum.tile([HID, 4 * 128], f32, tag="mm", name="mm",
                                space="PSUM", bufs=3)
                nc.tensor.matmul(out=mm2[:], lhsT=w2s[:], rhs=h1[:],
                                 start=True, stop=True)
                h2 = wt([HID, 4 * 128], bf16, "h2")
                nc.scalar.activation(h2[:], mm2[:], AF.Relu, bias=b2s[:, 0:1])

                mm3 = psum.tile([1, 4 * 128], f32, tag="mm3", name="mm3",
                                space="PSUM", bufs=2)
                nc.tensor.matmul(out=mm3[:], lhsT=w3s[:], rhs=h2[:],
                                 start=True, stop=True)
                res = wt([1, 4 * 128], f32, "res")
                nc.scalar.activation(res[:], mm3[:], AF.Identity,
                                     bias=b3s[0:1, 0:1])
                nc.sync.dma_start(
                    outv[:, g * S + bt * 512:g * S + (bt + 1) * 512], res[:])

    nc.finalize()
    return nc


def _get_nc():
    key = ("bfloat16", SIM_MODE)
    if key not in _BUILT:
        _BUILT[key] = _build_nc("bfloat16")
    return _BUILT[key]


def _build_patch_table(planes: np.ndarray, np_dt) -> np.ndarray:
    # planes [3, 32, 512, 512] -> PT [3*512*512, 128]
    p = planes.transpose(0, 2, 3, 1)  # [3, H, W, C]
    pt = np.zeros((3, RES, RES, 4, EMB), dtype=np.float32)
    pt[:, :, :, 0] = p
    pt[:, :, :-1, 1] = p[:, :, 1:]
    pt[:, :-1, :, 2] = p[:, 1:]
    pt[:, :-1, :-1, 3] = p[:, 1:, 1:]
    return np.ascontiguousarray(pt.reshape(3 * CELLS, 4 * EMB)).astype(np_dt)


def _floor_pix(c: np.ndarray) -> np.ndarray:
    # must match device: ACT (c*255.5 + bias) then int32 cast
    pm = c.astype(np.float32) * np.float32(255.5) + np.float32(_cast_bias())
    if SIM_MODE:
        return pm.astype(np.int32)  # numpy trunc, like CoreSim
    return np.rint(pm).astype(np.int32)  # HW rint


def kernel(**inputs: np.ndarray) -> np.ndarray:
    global LAST_RESULTS
    import ml_dtypes
    from concourse.bass_utils import run_bass_kernel_spmd

    coords = np.asarray(inputs["coordinates"], dtype=np.float32)
    planes = np.asarray(inputs["planes"], dtype=np.float32)
    bf = ml_dtypes.bfloat16
    pt = _build_patch_table(planes, bf)
    w0t = np.ascontiguousarray(inputs["w0"].T).astype(bf)
    w1t = np.ascontiguousarray(inputs["w1"].T).astype(bf)
    w2t = np.ascontiguousarray(inputs["w2"].T).astype(bf)
    w3t = np.ascontiguousarray(inputs["w3"].T).astype(bf)
    b0 = np.asarray(inputs["b0"], np.float32).reshape(HID, 1)
    b1 = np.asarray(inputs["b1"], np.float32).reshape(HID, 1)
    b2 = np.asarray(inputs["b2"], np.float32).reshape(1 * HID, 1)
    b3 = np.asarray(inputs["b3"], np.float32).reshape(1, 1)

    n = coords.shape[0]
    y0 = _floor_pix(coords[:, 1])
    z0 = _floor_pix(coords[:, 2])
    ab = np.clip(y0 >> 6, 0, BANDS - 1)
    bb = np.clip(z0 >> 6, 0, BANDS - 1)
    bucket = (ab * BANDS + bb).astype(np.int64)

    order = np.argsort(bucket, kind="stable")
    sorted_ids = order
    counts = np.bincount(bucket, minlength=NGRP)
    starts = np.concatenate(([0], np.cumsum(counts)))

    # slot_map[core, g, i] = original point id (or -1 for pad)
    slot_map = np.full((NCORES, NGRP, S), -1, dtype=np.int64)
    crdw = np.zeros((NCORES, NGRP, 128, 6 * SJ), np.float32)
    crdi = np.zeros((NCORES, NGRP, 128, 3 * NIC), np.float32)
    # device weight streams: y-coord per plane then x-coord per plane
    cmap = np.array([1, 2, 2, 0, 1, 0])

    for g in range(NGRP):
        ids = sorted_ids[starts[g]:starts[g + 1]]
        nk = len(ids)
        assert nk <= NCORES * S, f"bucket {g} overflow: {nk}"
        for c in range(NCORES):
            chunk = ids[c::NCORES]
            m = len(chunk)
            assert m <= S, f"core chunk overflow: {m}"
            slot_map[c, g, :m] = chunk
            if m == 0:
                # fabricate an in-band point (weights harmless, output unused)
                aa, zz = g // BANDS, g % BANDS
                fake = np.array([0.0,
                                 (64 * aa + 32) / 255.5 - 1.0,
                                 (64 * zz + 32) / 255.5 - 1.0], np.float32)
                pts = np.tile(fake, (S, 1))
            else:
                pts = coords[chunk]
                if m < S:
                    pts = np.concatenate(
                        [pts, np.tile(pts[0], (S - m, 1))], axis=0)
            # [p, s, j]: coord cmap[s] of point p + 128j
            pw = pts.reshape(SJ, 128, 3)[:, :, cmap]  # [j, p, s]
            crdw[c, g] = pw.transpose(1, 2, 0).reshape(128, 6 * SJ)
            arr = pts.reshape(NIC, 16, 3).transpose(1, 2, 0).reshape(
                16, 3 * NIC)  # [q, (c, t)]
            crdi[c, g] = np.tile(arr, (8, 1))

    in_maps = []
    for c in range(NCORES):
        in_maps.append({
            "pt": pt,
            "crdw": np.ascontiguousarray(crdw[c]),
            "crdi": np.ascontiguousarray(crdi[c]),
            "w0t": w0t, "w1t": w1t, "w2t": w2t, "w3t": w3t,
            "b0c": b0, "b1c": b1, "b2c": b2, "b3c": b3,
        })

    nc = _get_nc()
    LAST_RESULTS = run_bass_kernel_spmd(nc, in_maps, list(range(NCORES)))

    full = np.zeros(n, np.float32)
    for c in range(NCORES):
        o = np.asarray(LAST_RESULTS.results[c]["out"], np.float32).ravel()
        sm = slot_map[c].ravel()
        valid = sm >= 0
        full[sm[valid]] = o[valid]
    return full.reshape(1, n, 1).astype(np.float32)
